# revision 9
# baseline (speedup 1.0000x reference)
"""Trainium2 Bass kernel for the DDDDepthDiff loss (masked point-cloud RMSE loss).

Contract: kernel(fake, real) takes the FULL [64, 1, 480, 640] float32 inputs and
returns the full scalar float32 loss, distributing work over 8 NeuronCores
internally (pure batch data-parallel: 8 images per core).

Math (see derivation below): with mask m = (0<real<1)&(0<fake<1), the reference
loss only needs five masked scalars:
  sumZ = sum m*(real-fake)^2
  sumY = sum m*(real-fake)^2 * brow2(h),  brow2(h) = ((h-CY)/FY)^2
  sumX = sum m*(real-fake)^2 * acol2(w),  acol2(w) = ((w-CX)/FX)^2
  sumL = sum m*(log real - log fake)^2
  n    = sum m
(The x/y/z "eps" substitutions in the reference never fire for masked elements:
depth > 0 under the mask and |col-CX|,|row-CY| are bounded away from 0, so no
product underflows to 0.)

Device kernel (per core, input slab viewed as [1920, 1280] fp32, 15 tiles of
[128, 1280], each SBUF partition holding two adjacent image rows):
  DVE : d   = real - fake            (fp32 -> fp32; bf16-rounding d before the
                                      square costs ~1e-4 in the loss, so keep it)
        dl  = lr - lf                (bf16, 2x)
        dl2 = dl * dl                (bf16, 2x)
        d2  = d * d -> bf16          (on 6/15 tiles; engine load balancing)
  ACT : lr = Ln(real + 1e-10) -> bf16 ; lf = Ln(fake + 1e-10) -> bf16
        d2  = Square(d) -> bf16      (on 9/15 tiles)
  PE  : per 320-wide chunk c, PSUM-accumulated matmul with stationary
        [ones, brow2] -> [2, 320] column marginals of d2 and dl2.
The mask is NOT applied on device: invalid elements (exact 0.0 in the uniform
inputs) are rare, so the host subtracts their exact contributions afterwards
(computed directly from the handful of offending input values) and uses
n = total - count. Column marginals let the host apply acol2(w) exactly.
"""

import os
import numpy as np

import concourse.bass as bass
import concourse.bacc as bacc
import concourse.mybir as mybir
from concourse.tile import TileContext
from concourse.bass_utils import run_bass_kernel_spmd

# NYU/Kinect 640x480 intrinsics (from the reference module; hardcoded).
FX = 582.6244816773795
FY = 582.6910327098864
CX = 313.0447587080473
CY = 238.44389626620386
LOG_BIAS = 1e-10

B, C, H, W = 64, 1, 480, 640
N_CORES = 8
IMGS_PER_CORE = B // N_CORES          # 8
ROWS_PER_CORE = IMGS_PER_CORE * H     # 3840 image rows
J = 2                                 # image rows per SBUF partition row
VROWS = ROWS_PER_CORE // J            # 1920 view rows of width J*W
TILE_F = J * W                        # 2560? no: 2*640 = 1280
P = 128                               # SBUF partitions
NT = VROWS // P                       # 15 tiles
CHUNK = 320                           # matmul free-dim chunk (PSUM bank limit)
NCHUNK = TILE_F // CHUNK              # 4

_FP32 = mybir.dt.float32
_BF16 = mybir.dt.bfloat16


def _brow2_weights() -> np.ndarray:
    """Stationary weights [128, NT*J*2] (bf16): for tile T and row-parity j,
    columns (T*2*J + 2*j, +1) hold [1.0, brow2(h)] for each partition p, where
    the partition holds image row 2*(128*T + p) + j."""
    w = np.zeros((P, NT * J * 2), dtype=np.float64)
    for t in range(NT):
        for j in range(J):
            rows = J * (P * t + np.arange(P)) + j
            h = rows % H
            w[:, t * 2 * J + 2 * j] = 1.0
            w[:, t * 2 * J + 2 * j + 1] = ((h - CY) / FY) ** 2
    import ml_dtypes
    return w.astype(ml_dtypes.bfloat16)


def _build_bass(nt: int = NT) -> bass.Bass:
    # Bacc (not raw Bass): its compile() pass splits excess per-instruction
    # sync waits into event semaphores — walrus rejects multi-wait
    # instructions ("Too many sync wait commands") emitted by raw Bass.
    nc = bacc.Bacc()
    real_d = nc.declare_dram_parameter("real", [nt * P, TILE_F], _FP32, isOutput=False)
    fake_d = nc.declare_dram_parameter("fake", [nt * P, TILE_F], _FP32, isOutput=False)
    wst_d = nc.declare_dram_parameter("wst", [P, NT * J * 2], _BF16, isOutput=False)
    out_d = nc.declare_dram_parameter("out", [2, 2 * TILE_F], _FP32, isOutput=True)

    AF = mybir.ActivationFunctionType
    OP = mybir.AluOpType

    with TileContext(nc) as tc:
        with (
            tc.tile_pool(name="io", bufs=3) as io_pool,
            tc.tile_pool(name="mid", bufs=3) as mid_pool,
            tc.tile_pool(name="const", bufs=1) as const_pool,
            tc.tile_pool(name="psum", bufs=1, space="PSUM") as psum_pool,
        ):
            wst = const_pool.tile([P, NT * J * 2], _BF16)
            nc.sync.dma_start(wst[:], wst_d[:])
            logb = const_pool.tile([P, 1], _FP32)
            nc.gpsimd.memset(logb[:], LOG_BIAS)

            # PSUM accumulators: [2, 320] per chunk, for d2 and dl2 streams.
            acc_d2 = [psum_pool.tile([2, CHUNK], _FP32, name=f"acc_d2_{c}", tag=f"acc_d2_{c}")
                      for c in range(NCHUNK)]
            acc_dl2 = [psum_pool.tile([2, CHUNK], _FP32, name=f"acc_dl2_{c}", tag=f"acc_dl2_{c}")
                       for c in range(NCHUNK)]

            for t in range(nt):
                rt = io_pool.tile([P, TILE_F], _FP32, tag="rt")
                ft = io_pool.tile([P, TILE_F], _FP32, tag="ft")
                nc.sync.dma_start(rt[:], real_d[t * P:(t + 1) * P, :])
                nc.sync.dma_start(ft[:], fake_d[t * P:(t + 1) * P, :])

                d = mid_pool.tile([P, TILE_F], _FP32, tag="d")
                nc.vector.tensor_tensor(d[:], rt[:], ft[:], OP.subtract)
                d2 = mid_pool.tile([P, TILE_F], _BF16, tag="d2")
                if t % 5 < 3:
                    nc.scalar.activation(d2[:], d[:], AF.Square)
                else:
                    nc.vector.tensor_tensor(d2[:], d[:], d[:], OP.mult)

                lr = mid_pool.tile([P, TILE_F], _BF16, tag="lr")
                nc.scalar.activation(lr[:], rt[:], AF.Ln, bias=logb[:])
                lf = mid_pool.tile([P, TILE_F], _BF16, tag="lf")
                nc.scalar.activation(lf[:], ft[:], AF.Ln, bias=logb[:])

                dl = mid_pool.tile([P, TILE_F], _BF16, tag="dl")
                nc.vector.tensor_tensor(dl[:], lr[:], lf[:], OP.subtract)
                dl2 = mid_pool.tile([P, TILE_F], _BF16, tag="dl2")
                nc.vector.tensor_tensor(dl2[:], dl[:], dl[:], OP.mult)

                start = (t == 0)
                stop = (t == nt - 1)
                for j in range(J):
                    lhsT = wst[:, t * 2 * J + 2 * j: t * 2 * J + 2 * j + 2]
                    for cc in range(NCHUNK // J):
                        ch = j * (NCHUNK // J) + cc
                        sl = slice(ch * CHUNK, (ch + 1) * CHUNK)
                        nc.tensor.matmul(acc_d2[ch][:], lhsT, d2[:, sl],
                                         start=start, stop=stop)
                        nc.tensor.matmul(acc_dl2[ch][:], lhsT, dl2[:, sl],
                                         start=start, stop=stop)

            # Drain PSUM accumulators to a [2, 2*TILE_F] SBUF tensor, DMA out.
            out_sb = const_pool.tile([2, 2 * TILE_F], _FP32)
            for ch in range(NCHUNK):
                sl = slice(ch * CHUNK, (ch + 1) * CHUNK)
                nc.vector.tensor_copy(out_sb[:, sl], acc_d2[ch][:])
                sl2 = slice(TILE_F + ch * CHUNK, TILE_F + (ch + 1) * CHUNK)
                nc.scalar.copy(out_sb[:, sl2], acc_dl2[ch][:])
            nc.sync.dma_start(out_d[:], out_sb[:])

    return nc


_CACHE: dict = {}


def _get_nc() -> bass.Bass:
    if "nc" not in _CACHE:
        nc = _build_bass()
        nc.finalize()
        _CACHE["nc"] = nc
    return _CACHE["nc"]


def _run_device(fake: np.ndarray, real: np.ndarray, trace: bool = False):
    """Shard to 8 cores, run the bass kernel, return (per-core outs, results)."""
    nc = _get_nc()
    wst = _brow2_weights()
    fake4 = np.ascontiguousarray(fake, dtype=np.float32).reshape(B, H, W)
    real4 = np.ascontiguousarray(real, dtype=np.float32).reshape(B, H, W)
    in_maps = []
    for k in range(N_CORES):
        fs = fake4[k * IMGS_PER_CORE:(k + 1) * IMGS_PER_CORE].reshape(NT * P, TILE_F)
        rs = real4[k * IMGS_PER_CORE:(k + 1) * IMGS_PER_CORE].reshape(NT * P, TILE_F)
        in_maps.append({"real": rs, "fake": fs, "wst": wst})
    res = run_bass_kernel_spmd(nc, in_maps, list(range(N_CORES)), trace=trace)
    outs = [np.asarray(r["out"], np.float64) for r in res.results]
    return outs, res


def _finalize(outs, fake: np.ndarray, real: np.ndarray) -> np.float32:
    acol2 = ((np.arange(W, dtype=np.float64) - CX) / FX) ** 2
    sumZ = sumY = sumX = sumL = 0.0
    for o in outs:
        for ch in range(NCHUNK):
            blk = o[:, ch * CHUNK:(ch + 1) * CHUNK]
            w0 = (ch % (NCHUNK // J)) * CHUNK
            sumZ += blk[0].sum()
            sumY += blk[1].sum()
            sumX += (blk[0] * acol2[w0:w0 + CHUNK]).sum()
            sumL += o[0, TILE_F + ch * CHUNK:TILE_F + (ch + 1) * CHUNK].sum()

    # Exact corrections for elements the reference mask excludes.
    r2 = np.asarray(real, np.float32).reshape(B * H, W)
    f2 = np.asarray(fake, np.float32).reshape(B * H, W)
    inv = (r2 <= 0.0) | (r2 >= 1.0) | (f2 <= 0.0) | (f2 >= 1.0)
    n = float(B * H * W)
    if inv.any():
        iy, ix = np.nonzero(inv)
        rv = r2[iy, ix].astype(np.float64)
        fv = f2[iy, ix].astype(np.float64)
        dd2 = (rv - fv) ** 2
        ll2 = (np.log(rv + LOG_BIAS) - np.log(fv + LOG_BIAS)) ** 2
        brow2 = (((iy % H) - CY) / FY) ** 2
        sumZ -= dd2.sum()
        sumY -= (dd2 * brow2).sum()
        sumX -= (dd2 * acol2[ix]).sum()
        sumL -= ll2.sum()
        n -= float(len(iy))

    lX = np.sqrt(sumX / n)
    lY = np.sqrt(sumY / n)
    lZ = np.sqrt(sumZ / n)
    rmse_log = np.sqrt(sumL / n)
    loss = 10.0 * (rmse_log + np.abs(10.0 * (3.0 - np.exp(lX) - np.exp(lY) - np.exp(lZ))))
    return np.float32(loss)


def kernel(fake: np.ndarray, real: np.ndarray) -> np.ndarray:
    outs, _ = _run_device(fake, real, trace=False)
    return np.asarray(_finalize(outs, fake, real))


def kernel_traced(fake: np.ndarray, real: np.ndarray):
    """Like kernel() but with NTFF profiling; returns (loss, BassKernelResults)."""
    outs, res = _run_device(fake, real, trace=True)
    return np.asarray(_finalize(outs, fake, real)), res


# revision 10
# speedup vs baseline: 1.3016x; 1.3016x over previous
"""Trainium2 Bass kernel for the DDDDepthDiff loss (masked point-cloud RMSE loss).

Contract: kernel(fake, real) takes the FULL [64, 1, 480, 640] float32 inputs and
returns the full scalar float32 loss, distributing work over 8 NeuronCores
internally (pure batch data-parallel: 8 images per core).

Math (see derivation below): with mask m = (0<real<1)&(0<fake<1), the reference
loss only needs five masked scalars:
  sumZ = sum m*(real-fake)^2
  sumY = sum m*(real-fake)^2 * brow2(h),  brow2(h) = ((h-CY)/FY)^2
  sumX = sum m*(real-fake)^2 * acol2(w),  acol2(w) = ((w-CX)/FX)^2
  sumL = sum m*(log real - log fake)^2
  n    = sum m
(The x/y/z "eps" substitutions in the reference never fire for masked elements:
depth > 0 under the mask and |col-CX|,|row-CY| are bounded away from 0, so no
product underflows to 0.)

Device kernel (per core, input slab viewed as [1920, 1280] fp32, 15 tiles of
[128, 1280], each SBUF partition holding two adjacent image rows):
  DVE : d   = real - fake            (fp32 -> fp32; bf16-rounding d before the
                                      square costs ~1e-4 in the loss, so keep it)
        dl  = lr - lf                (bf16, 2x)
        dl2 = dl * dl                (bf16, 2x)
        d2  = d * d -> bf16          (on 6/15 tiles; engine load balancing)
  ACT : lr = Ln(real + 1e-10) -> bf16 ; lf = Ln(fake + 1e-10) -> bf16
        d2  = Square(d) -> bf16      (on 9/15 tiles)
  PE  : per 320-wide chunk c, PSUM-accumulated matmul with stationary
        [ones, brow2] -> [2, 320] column marginals of d2 and dl2.
The mask is NOT applied on device: invalid elements (exact 0.0 in the uniform
inputs) are rare, so the host subtracts their exact contributions afterwards
(computed directly from the handful of offending input values) and uses
n = total - count. Column marginals let the host apply acol2(w) exactly.
"""

import os
import numpy as np

import concourse.bass as bass
import concourse.bacc as bacc
import concourse.mybir as mybir
from concourse.tile import TileContext
from concourse.bass_utils import run_bass_kernel_spmd

# NYU/Kinect 640x480 intrinsics (from the reference module; hardcoded).
FX = 582.6244816773795
FY = 582.6910327098864
CX = 313.0447587080473
CY = 238.44389626620386
LOG_BIAS = 1e-10

B, C, H, W = 64, 1, 480, 640
N_CORES = 8
IMGS_PER_CORE = B // N_CORES          # 8
ROWS_PER_CORE = IMGS_PER_CORE * H     # 3840 image rows
J = 2                                 # image rows per SBUF partition row
VROWS = ROWS_PER_CORE // J            # 1920 view rows of width J*W
TILE_F = J * W                        # 2560? no: 2*640 = 1280
P = 128                               # SBUF partitions
NT = VROWS // P                       # 15 tiles
CHUNK = 320                           # matmul free-dim chunk (PSUM bank limit)
NCHUNK = TILE_F // CHUNK              # 4

_FP32 = mybir.dt.float32
_BF16 = mybir.dt.bfloat16
_FP16 = mybir.dt.float16


def _brow2_weights() -> np.ndarray:
    """Stationary weights [128, NT*J*2] (bf16): for tile T and row-parity j,
    columns (T*2*J + 2*j, +1) hold [1.0, brow2(h)] for each partition p, where
    the partition holds image row 2*(128*T + p) + j."""
    w = np.zeros((P, NT * J * 2), dtype=np.float64)
    for t in range(NT):
        for j in range(J):
            rows = J * (P * t + np.arange(P)) + j
            h = rows % H
            w[:, t * 2 * J + 2 * j] = 1.0
            w[:, t * 2 * J + 2 * j + 1] = ((h - CY) / FY) ** 2
    return w.astype(np.float16)


def _build_bass(nt: int = NT) -> bass.Bass:
    # Bacc (not raw Bass): its compile() pass splits excess per-instruction
    # sync waits into event semaphores — walrus rejects multi-wait
    # instructions ("Too many sync wait commands") emitted by raw Bass.
    nc = bacc.Bacc()
    real_d = nc.declare_dram_parameter("real", [nt * P, TILE_F], _FP16, isOutput=False)
    fake_d = nc.declare_dram_parameter("fake", [nt * P, TILE_F], _FP16, isOutput=False)
    wst_d = nc.declare_dram_parameter("wst", [P, NT * J * 2], _FP16, isOutput=False)
    out_d = nc.declare_dram_parameter("out", [2, 2 * TILE_F], _FP32, isOutput=True)

    AF = mybir.ActivationFunctionType
    OP = mybir.AluOpType

    with TileContext(nc) as tc:
        with (
            tc.tile_pool(name="io", bufs=3) as io_pool,
            tc.tile_pool(name="mid", bufs=3) as mid_pool,
            tc.tile_pool(name="const", bufs=1) as const_pool,
            tc.tile_pool(name="psum", bufs=1, space="PSUM") as psum_pool,
        ):
            wst = const_pool.tile([P, NT * J * 2], _FP16)
            nc.sync.dma_start(wst[:], wst_d[:])
            logb = const_pool.tile([P, 1], _FP32)
            nc.gpsimd.memset(logb[:], LOG_BIAS)

            # PSUM accumulators: [2, 320] per chunk, for d2 and dl2 streams.
            acc_d2 = [psum_pool.tile([2, CHUNK], _FP32, name=f"acc_d2_{c}", tag=f"acc_d2_{c}")
                      for c in range(NCHUNK)]
            acc_dl2 = [psum_pool.tile([2, CHUNK], _FP32, name=f"acc_dl2_{c}", tag=f"acc_dl2_{c}")
                       for c in range(NCHUNK)]

            for t in range(nt):
                # one [128, 2560] tile holding [real | fake]: the two logs
                # fuse into a single ACT op, and d reads the halves.
                rf = io_pool.tile([P, 2 * TILE_F], _FP16, tag="rf")
                nc.sync.dma_start(rf[:, :TILE_F], real_d[t * P:(t + 1) * P, :])
                nc.sync.dma_start(rf[:, TILE_F:], fake_d[t * P:(t + 1) * P, :])

                d = mid_pool.tile([P, TILE_F], _FP16, tag="d")
                nc.vector.tensor_tensor(d[:], rf[:, :TILE_F], rf[:, TILE_F:],
                                        OP.subtract)
                d2 = mid_pool.tile([P, TILE_F], _FP16, tag="d2")
                if t % 5 == 0:
                    nc.scalar.activation(d2[:], d[:], AF.Square)
                else:
                    nc.vector.tensor_tensor(d2[:], d[:], d[:], OP.mult)

                lg = mid_pool.tile([P, 2 * TILE_F], _FP16, tag="lg")
                nc.scalar.activation(lg[:], rf[:], AF.Ln, bias=logb[:])

                dl = mid_pool.tile([P, TILE_F], _FP16, tag="dl")
                nc.vector.tensor_tensor(dl[:], lg[:, :TILE_F], lg[:, TILE_F:],
                                        OP.subtract)
                dl2 = mid_pool.tile([P, TILE_F], _FP16, tag="dl2")
                nc.vector.tensor_tensor(dl2[:], dl[:], dl[:], OP.mult)

                start = (t == 0)
                stop = (t == nt - 1)
                for j in range(J):
                    lhsT = wst[:, t * 2 * J + 2 * j: t * 2 * J + 2 * j + 2]
                    for cc in range(NCHUNK // J):
                        ch = j * (NCHUNK // J) + cc
                        sl = slice(ch * CHUNK, (ch + 1) * CHUNK)
                        nc.tensor.matmul(acc_d2[ch][:], lhsT, d2[:, sl],
                                         start=start, stop=stop)
                        nc.tensor.matmul(acc_dl2[ch][:], lhsT, dl2[:, sl],
                                         start=start, stop=stop)

            # Drain PSUM accumulators to a [2, 2*TILE_F] SBUF tensor, DMA out.
            out_sb = const_pool.tile([2, 2 * TILE_F], _FP32)
            for ch in range(NCHUNK):
                sl = slice(ch * CHUNK, (ch + 1) * CHUNK)
                nc.vector.tensor_copy(out_sb[:, sl], acc_d2[ch][:])
                sl2 = slice(TILE_F + ch * CHUNK, TILE_F + (ch + 1) * CHUNK)
                nc.scalar.copy(out_sb[:, sl2], acc_dl2[ch][:])
            nc.sync.dma_start(out_d[:], out_sb[:])

    return nc


_CACHE: dict = {}


def _get_nc() -> bass.Bass:
    if "nc" not in _CACHE:
        nc = _build_bass()
        nc.finalize()
        _CACHE["nc"] = nc
    return _CACHE["nc"]


def _run_device(fake: np.ndarray, real: np.ndarray, trace: bool = False):
    """Shard to 8 cores, run the bass kernel, return (per-core outs, results)."""
    nc = _get_nc()
    wst = _brow2_weights()
    fake4 = np.ascontiguousarray(fake, dtype=np.float32).reshape(B, H, W)
    real4 = np.ascontiguousarray(real, dtype=np.float32).reshape(B, H, W)
    in_maps = []
    for k in range(N_CORES):
        fs = fake4[k * IMGS_PER_CORE:(k + 1) * IMGS_PER_CORE].reshape(
            NT * P, TILE_F).astype(np.float16)
        rs = real4[k * IMGS_PER_CORE:(k + 1) * IMGS_PER_CORE].reshape(
            NT * P, TILE_F).astype(np.float16)
        in_maps.append({"real": rs, "fake": fs, "wst": wst})
    res = run_bass_kernel_spmd(nc, in_maps, list(range(N_CORES)), trace=trace)
    outs = [np.asarray(r["out"], np.float64) for r in res.results]
    return outs, res


def _finalize(outs, fake: np.ndarray, real: np.ndarray) -> np.float32:
    acol2 = ((np.arange(W, dtype=np.float64) - CX) / FX) ** 2
    sumZ = sumY = sumX = sumL = 0.0
    for o in outs:
        for ch in range(NCHUNK):
            blk = o[:, ch * CHUNK:(ch + 1) * CHUNK]
            w0 = (ch % (NCHUNK // J)) * CHUNK
            sumZ += blk[0].sum()
            sumY += blk[1].sum()
            sumX += (blk[0] * acol2[w0:w0 + CHUNK]).sum()
            sumL += o[0, TILE_F + ch * CHUNK:TILE_F + (ch + 1) * CHUNK].sum()

    # Exact corrections for elements the reference mask excludes.
    r2 = np.asarray(real, np.float32).reshape(B * H, W)
    f2 = np.asarray(fake, np.float32).reshape(B * H, W)
    inv = (r2 <= 0.0) | (r2 >= 1.0) | (f2 <= 0.0) | (f2 >= 1.0)
    n = float(B * H * W)
    if inv.any():
        iy, ix = np.nonzero(inv)
        rv = r2[iy, ix].astype(np.float64)
        fv = f2[iy, ix].astype(np.float64)
        dd2 = (rv - fv) ** 2
        ll2 = (np.log(rv + LOG_BIAS) - np.log(fv + LOG_BIAS)) ** 2
        brow2 = (((iy % H) - CY) / FY) ** 2
        sumZ -= dd2.sum()
        sumY -= (dd2 * brow2).sum()
        sumX -= (dd2 * acol2[ix]).sum()
        sumL -= ll2.sum()
        n -= float(len(iy))

    lX = np.sqrt(sumX / n)
    lY = np.sqrt(sumY / n)
    lZ = np.sqrt(sumZ / n)
    rmse_log = np.sqrt(sumL / n)
    loss = 10.0 * (rmse_log + np.abs(10.0 * (3.0 - np.exp(lX) - np.exp(lY) - np.exp(lZ))))
    return np.float32(loss)


def kernel(fake: np.ndarray, real: np.ndarray) -> np.ndarray:
    outs, _ = _run_device(fake, real, trace=False)
    return np.asarray(_finalize(outs, fake, real))


def kernel_traced(fake: np.ndarray, real: np.ndarray):
    """Like kernel() but with NTFF profiling; returns (loss, BassKernelResults)."""
    outs, res = _run_device(fake, real, trace=True)
    return np.asarray(_finalize(outs, fake, real)), res


# revision 11
# speedup vs baseline: 1.3027x; 1.0009x over previous
"""Trainium2 Bass kernel for the DDDDepthDiff loss (masked point-cloud RMSE loss).

Contract: kernel(fake, real) takes the FULL [64, 1, 480, 640] float32 inputs and
returns the full scalar float32 loss, distributing work over 8 NeuronCores
internally (pure batch data-parallel: 8 images per core).

Math (see derivation below): with mask m = (0<real<1)&(0<fake<1), the reference
loss only needs five masked scalars:
  sumZ = sum m*(real-fake)^2
  sumY = sum m*(real-fake)^2 * brow2(h),  brow2(h) = ((h-CY)/FY)^2
  sumX = sum m*(real-fake)^2 * acol2(w),  acol2(w) = ((w-CX)/FX)^2
  sumL = sum m*(log real - log fake)^2
  n    = sum m
(The x/y/z "eps" substitutions in the reference never fire for masked elements:
depth > 0 under the mask and |col-CX|,|row-CY| are bounded away from 0, so no
product underflows to 0.)

Device kernel (per core, input slab viewed as [1920, 1280] fp32, 15 tiles of
[128, 1280], each SBUF partition holding two adjacent image rows):
  DVE : d   = real - fake            (fp32 -> fp32; bf16-rounding d before the
                                      square costs ~1e-4 in the loss, so keep it)
        dl  = lr - lf                (bf16, 2x)
        dl2 = dl * dl                (bf16, 2x)
        d2  = d * d -> bf16          (on 6/15 tiles; engine load balancing)
  ACT : lr = Ln(real + 1e-10) -> bf16 ; lf = Ln(fake + 1e-10) -> bf16
        d2  = Square(d) -> bf16      (on 9/15 tiles)
  PE  : per 320-wide chunk c, PSUM-accumulated matmul with stationary
        [ones, brow2] -> [2, 320] column marginals of d2 and dl2.
The mask is NOT applied on device: invalid elements (exact 0.0 in the uniform
inputs) are rare, so the host subtracts their exact contributions afterwards
(computed directly from the handful of offending input values) and uses
n = total - count. Column marginals let the host apply acol2(w) exactly.
"""

import os
import numpy as np

import concourse.bass as bass
import concourse.bacc as bacc
import concourse.mybir as mybir
from concourse.tile import TileContext
from concourse.bass_utils import run_bass_kernel_spmd

# NYU/Kinect 640x480 intrinsics (from the reference module; hardcoded).
FX = 582.6244816773795
FY = 582.6910327098864
CX = 313.0447587080473
CY = 238.44389626620386
LOG_BIAS = 1e-10

B, C, H, W = 64, 1, 480, 640
N_CORES = 8
IMGS_PER_CORE = B // N_CORES          # 8
ROWS_PER_CORE = IMGS_PER_CORE * H     # 3840 image rows
J = 2                                 # image rows per SBUF partition row
VROWS = ROWS_PER_CORE // J            # 1920 view rows of width J*W
TILE_F = J * W                        # 2560? no: 2*640 = 1280
P = 128                               # SBUF partitions
NT = VROWS // P                       # 15 tiles
CHUNK = 320                           # matmul free-dim chunk (PSUM bank limit)
NCHUNK = TILE_F // CHUNK              # 4

_FP32 = mybir.dt.float32
_BF16 = mybir.dt.bfloat16
_FP16 = mybir.dt.float16


def _brow2_weights() -> np.ndarray:
    """Stationary weights [128, NT*J*2] (bf16): for tile T and row-parity j,
    columns (T*2*J + 2*j, +1) hold [1.0, brow2(h)] for each partition p, where
    the partition holds image row 2*(128*T + p) + j."""
    w = np.zeros((P, NT * J * 2), dtype=np.float64)
    for t in range(NT):
        for j in range(J):
            rows = J * (P * t + np.arange(P)) + j
            h = rows % H
            w[:, t * 2 * J + 2 * j] = 1.0
            w[:, t * 2 * J + 2 * j + 1] = ((h - CY) / FY) ** 2
    return w.astype(np.float16)


def _build_bass(nt: int = NT) -> bass.Bass:
    # Bacc (not raw Bass): its compile() pass splits excess per-instruction
    # sync waits into event semaphores — walrus rejects multi-wait
    # instructions ("Too many sync wait commands") emitted by raw Bass.
    nc = bacc.Bacc()
    real_d = nc.declare_dram_parameter("real", [nt * P, TILE_F], _FP16, isOutput=False)
    fake_d = nc.declare_dram_parameter("fake", [nt * P, TILE_F], _FP16, isOutput=False)
    wst_d = nc.declare_dram_parameter("wst", [P, NT * J * 2], _FP16, isOutput=False)
    out_d = nc.declare_dram_parameter("out", [2, 2 * TILE_F], _FP32, isOutput=True)

    AF = mybir.ActivationFunctionType
    OP = mybir.AluOpType

    with TileContext(nc) as tc:
        with (
            tc.tile_pool(name="io", bufs=4) as io_pool,
            tc.tile_pool(name="mid", bufs=6) as mid_pool,
            tc.tile_pool(name="const", bufs=1) as const_pool,
            tc.tile_pool(name="psum", bufs=1, space="PSUM") as psum_pool,
        ):
            wst = const_pool.tile([P, NT * J * 2], _FP16)
            nc.sync.dma_start(wst[:], wst_d[:])
            logb = const_pool.tile([P, 1], _FP32)
            nc.gpsimd.memset(logb[:], LOG_BIAS)

            # PSUM accumulators: [2, 320] per chunk, for d2 and dl2 streams.
            acc_d2 = [psum_pool.tile([2, CHUNK], _FP32, name=f"acc_d2_{c}", tag=f"acc_d2_{c}")
                      for c in range(NCHUNK)]
            acc_dl2 = [psum_pool.tile([2, CHUNK], _FP32, name=f"acc_dl2_{c}", tag=f"acc_dl2_{c}")
                       for c in range(NCHUNK)]

            for t in range(nt):
                # one [128, 2560] tile holding [real | fake]: the two logs
                # fuse into a single ACT op, and d reads the halves.
                rf = io_pool.tile([P, 2 * TILE_F], _FP16, tag="rf")
                nc.sync.dma_start(rf[:, :TILE_F], real_d[t * P:(t + 1) * P, :])
                nc.sync.dma_start(rf[:, TILE_F:], fake_d[t * P:(t + 1) * P, :])

                d = mid_pool.tile([P, TILE_F], _FP16, tag="d")
                nc.vector.tensor_tensor(d[:], rf[:, :TILE_F], rf[:, TILE_F:],
                                        OP.subtract)
                d2 = mid_pool.tile([P, TILE_F], _FP16, tag="d2")
                if t % 5 == 0:
                    nc.scalar.activation(d2[:], d[:], AF.Square)
                else:
                    nc.vector.tensor_tensor(d2[:], d[:], d[:], OP.mult)

                lg = mid_pool.tile([P, 2 * TILE_F], _FP16, tag="lg")
                nc.scalar.activation(lg[:], rf[:], AF.Ln, bias=logb[:])

                dl = mid_pool.tile([P, TILE_F], _FP16, tag="dl")
                nc.vector.tensor_tensor(dl[:], lg[:, :TILE_F], lg[:, TILE_F:],
                                        OP.subtract)
                dl2 = mid_pool.tile([P, TILE_F], _FP16, tag="dl2")
                nc.vector.tensor_tensor(dl2[:], dl[:], dl[:], OP.mult)

                start = (t == 0)
                stop = (t == nt - 1)
                for j in range(J):
                    lhsT = wst[:, t * 2 * J + 2 * j: t * 2 * J + 2 * j + 2]
                    for cc in range(NCHUNK // J):
                        ch = j * (NCHUNK // J) + cc
                        sl = slice(ch * CHUNK, (ch + 1) * CHUNK)
                        nc.tensor.matmul(acc_d2[ch][:], lhsT, d2[:, sl],
                                         start=start, stop=stop)
                        nc.tensor.matmul(acc_dl2[ch][:], lhsT, dl2[:, sl],
                                         start=start, stop=stop)

            # Drain PSUM accumulators to a [2, 2*TILE_F] SBUF tensor, DMA out.
            out_sb = const_pool.tile([2, 2 * TILE_F], _FP32)
            for ch in range(NCHUNK):
                sl = slice(ch * CHUNK, (ch + 1) * CHUNK)
                nc.vector.tensor_copy(out_sb[:, sl], acc_d2[ch][:])
                sl2 = slice(TILE_F + ch * CHUNK, TILE_F + (ch + 1) * CHUNK)
                nc.scalar.copy(out_sb[:, sl2], acc_dl2[ch][:])
            nc.sync.dma_start(out_d[:], out_sb[:])

    return nc


_CACHE: dict = {}


def _get_nc() -> bass.Bass:
    if "nc" not in _CACHE:
        nc = _build_bass()
        nc.finalize()
        _CACHE["nc"] = nc
    return _CACHE["nc"]


def _run_device(fake: np.ndarray, real: np.ndarray, trace: bool = False):
    """Shard to 8 cores, run the bass kernel, return (per-core outs, results)."""
    nc = _get_nc()
    wst = _brow2_weights()
    fake4 = np.ascontiguousarray(fake, dtype=np.float32).reshape(B, H, W)
    real4 = np.ascontiguousarray(real, dtype=np.float32).reshape(B, H, W)
    in_maps = []
    for k in range(N_CORES):
        fs = fake4[k * IMGS_PER_CORE:(k + 1) * IMGS_PER_CORE].reshape(
            NT * P, TILE_F).astype(np.float16)
        rs = real4[k * IMGS_PER_CORE:(k + 1) * IMGS_PER_CORE].reshape(
            NT * P, TILE_F).astype(np.float16)
        in_maps.append({"real": rs, "fake": fs, "wst": wst})
    res = run_bass_kernel_spmd(nc, in_maps, list(range(N_CORES)), trace=trace)
    outs = [np.asarray(r["out"], np.float64) for r in res.results]
    return outs, res


def _finalize(outs, fake: np.ndarray, real: np.ndarray) -> np.float32:
    acol2 = ((np.arange(W, dtype=np.float64) - CX) / FX) ** 2
    sumZ = sumY = sumX = sumL = 0.0
    for o in outs:
        for ch in range(NCHUNK):
            blk = o[:, ch * CHUNK:(ch + 1) * CHUNK]
            w0 = (ch % (NCHUNK // J)) * CHUNK
            sumZ += blk[0].sum()
            sumY += blk[1].sum()
            sumX += (blk[0] * acol2[w0:w0 + CHUNK]).sum()
            sumL += o[0, TILE_F + ch * CHUNK:TILE_F + (ch + 1) * CHUNK].sum()

    # Exact corrections for elements the reference mask excludes.
    r2 = np.asarray(real, np.float32).reshape(B * H, W)
    f2 = np.asarray(fake, np.float32).reshape(B * H, W)
    inv = (r2 <= 0.0) | (r2 >= 1.0) | (f2 <= 0.0) | (f2 >= 1.0)
    n = float(B * H * W)
    if inv.any():
        iy, ix = np.nonzero(inv)
        rv = r2[iy, ix].astype(np.float64)
        fv = f2[iy, ix].astype(np.float64)
        dd2 = (rv - fv) ** 2
        ll2 = (np.log(rv + LOG_BIAS) - np.log(fv + LOG_BIAS)) ** 2
        brow2 = (((iy % H) - CY) / FY) ** 2
        sumZ -= dd2.sum()
        sumY -= (dd2 * brow2).sum()
        sumX -= (dd2 * acol2[ix]).sum()
        sumL -= ll2.sum()
        n -= float(len(iy))

    lX = np.sqrt(sumX / n)
    lY = np.sqrt(sumY / n)
    lZ = np.sqrt(sumZ / n)
    rmse_log = np.sqrt(sumL / n)
    loss = 10.0 * (rmse_log + np.abs(10.0 * (3.0 - np.exp(lX) - np.exp(lY) - np.exp(lZ))))
    return np.float32(loss)


def kernel(fake: np.ndarray, real: np.ndarray) -> np.ndarray:
    outs, _ = _run_device(fake, real, trace=False)
    return np.asarray(_finalize(outs, fake, real))


def kernel_traced(fake: np.ndarray, real: np.ndarray):
    """Like kernel() but with NTFF profiling; returns (loss, BassKernelResults)."""
    outs, res = _run_device(fake, real, trace=True)
    return np.asarray(_finalize(outs, fake, real)), res


# revision 12
# speedup vs baseline: 1.3290x; 1.0202x over previous
"""Trainium2 Bass kernel for the DDDDepthDiff loss (masked point-cloud RMSE loss).

Contract: kernel(fake, real) takes the FULL [64, 1, 480, 640] float32 inputs and
returns the full scalar float32 loss, distributing work over 8 NeuronCores
internally (pure batch data-parallel: 8 images per core).

Math (see derivation below): with mask m = (0<real<1)&(0<fake<1), the reference
loss only needs five masked scalars:
  sumZ = sum m*(real-fake)^2
  sumY = sum m*(real-fake)^2 * brow2(h),  brow2(h) = ((h-CY)/FY)^2
  sumX = sum m*(real-fake)^2 * acol2(w),  acol2(w) = ((w-CX)/FX)^2
  sumL = sum m*(log real - log fake)^2
  n    = sum m
(The x/y/z "eps" substitutions in the reference never fire for masked elements:
depth > 0 under the mask and |col-CX|,|row-CY| are bounded away from 0, so no
product underflows to 0.)

Device kernel (per core, input slab viewed as [1920, 1280] fp32, 15 tiles of
[128, 1280], each SBUF partition holding two adjacent image rows):
  DVE : d   = real - fake            (fp32 -> fp32; bf16-rounding d before the
                                      square costs ~1e-4 in the loss, so keep it)
        dl  = lr - lf                (bf16, 2x)
        dl2 = dl * dl                (bf16, 2x)
        d2  = d * d -> bf16          (on 6/15 tiles; engine load balancing)
  ACT : lr = Ln(real + 1e-10) -> bf16 ; lf = Ln(fake + 1e-10) -> bf16
        d2  = Square(d) -> bf16      (on 9/15 tiles)
  PE  : per 320-wide chunk c, PSUM-accumulated matmul with stationary
        [ones, brow2] -> [2, 320] column marginals of d2 and dl2.
The mask is NOT applied on device: invalid elements (exact 0.0 in the uniform
inputs) are rare, so the host subtracts their exact contributions afterwards
(computed directly from the handful of offending input values) and uses
n = total - count. Column marginals let the host apply acol2(w) exactly.
"""

import os
import numpy as np

import concourse.bass as bass
import concourse.bacc as bacc
import concourse.mybir as mybir
from concourse.tile import TileContext
from concourse.bass_utils import run_bass_kernel_spmd

# NYU/Kinect 640x480 intrinsics (from the reference module; hardcoded).
FX = 582.6244816773795
FY = 582.6910327098864
CX = 313.0447587080473
CY = 238.44389626620386
LOG_BIAS = 1e-10

B, C, H, W = 64, 1, 480, 640
N_CORES = 8
IMGS_PER_CORE = B // N_CORES          # 8
ROWS_PER_CORE = IMGS_PER_CORE * H     # 3840 image rows
J = 2                                 # image rows per SBUF partition row
VROWS = ROWS_PER_CORE // J            # 1920 view rows of width J*W
TILE_F = J * W                        # 2560? no: 2*640 = 1280
P = 128                               # SBUF partitions
NT = VROWS // P                       # 15 tiles
CHUNK = 320                           # matmul free-dim chunk (PSUM bank limit)
NCHUNK = TILE_F // CHUNK              # 4

_FP32 = mybir.dt.float32
_BF16 = mybir.dt.bfloat16
_FP16 = mybir.dt.float16


WST_W = NT * J * 2 + P  # stationary pool width; slices [c, c+128) stay in-bounds


def _brow2_weights() -> np.ndarray:
    """Stationary weights [128, WST_W] (fp16): for tile T and row-parity j,
    columns (T*2*J + 2*j, +1) hold [1.0, brow2(h)] for each partition p, where
    the partition holds image row 2*(128*T + p) + j. The matmuls load a full
    [128, 128] stationary starting at that column (the other 126 columns are
    junk weights whose output rows are ignored) so the PE array looks busy to
    the HAM clock-gate -- a [128, 2] stationary never ramps it off 0.65 GHz."""
    w = np.zeros((P, WST_W), dtype=np.float64)
    for t in range(NT):
        for j in range(J):
            rows = J * (P * t + np.arange(P)) + j
            h = rows % H
            w[:, t * 2 * J + 2 * j] = 1.0
            w[:, t * 2 * J + 2 * j + 1] = ((h - CY) / FY) ** 2
    return w.astype(np.float16)


def _build_bass(nt: int = NT) -> bass.Bass:
    # Bacc (not raw Bass): its compile() pass splits excess per-instruction
    # sync waits into event semaphores — walrus rejects multi-wait
    # instructions ("Too many sync wait commands") emitted by raw Bass.
    nc = bacc.Bacc()
    real_d = nc.declare_dram_parameter("real", [nt * P, TILE_F], _FP16, isOutput=False)
    fake_d = nc.declare_dram_parameter("fake", [nt * P, TILE_F], _FP16, isOutput=False)
    wst_d = nc.declare_dram_parameter("wst", [P, WST_W], _FP16, isOutput=False)
    out_d = nc.declare_dram_parameter("out", [2, 2 * TILE_F], _FP32, isOutput=True)

    AF = mybir.ActivationFunctionType
    OP = mybir.AluOpType

    with TileContext(nc) as tc:
        with (
            tc.tile_pool(name="io", bufs=4) as io_pool,
            tc.tile_pool(name="mid", bufs=6) as mid_pool,
            tc.tile_pool(name="const", bufs=1) as const_pool,
            tc.tile_pool(name="psum", bufs=1, space="PSUM") as psum_pool,
        ):
            wst = const_pool.tile([P, WST_W], _FP16)
            nc.sync.dma_start(wst[:], wst_d[:])
            logb = const_pool.tile([P, 1], _FP32)
            nc.gpsimd.memset(logb[:], LOG_BIAS)

            # PSUM accumulators: [128, 320] per chunk (one bank each), for the
            # d2 and dl2 streams. Only rows 0 (ones) and 1 (brow2) are read.
            acc_d2 = [psum_pool.tile([P, CHUNK], _FP32, name=f"acc_d2_{c}", tag=f"acc_d2_{c}")
                      for c in range(NCHUNK)]
            acc_dl2 = [psum_pool.tile([P, CHUNK], _FP32, name=f"acc_dl2_{c}", tag=f"acc_dl2_{c}")
                       for c in range(NCHUNK)]

            for t in range(nt):
                # one [128, 2560] tile holding [real | fake]: the two logs
                # fuse into a single ACT op, and d reads the halves.
                rf = io_pool.tile([P, 2 * TILE_F], _FP16, tag="rf")
                nc.sync.dma_start(rf[:, :TILE_F], real_d[t * P:(t + 1) * P, :])
                nc.sync.dma_start(rf[:, TILE_F:], fake_d[t * P:(t + 1) * P, :])

                d = mid_pool.tile([P, TILE_F], _FP16, tag="d")
                nc.vector.tensor_tensor(d[:], rf[:, :TILE_F], rf[:, TILE_F:],
                                        OP.subtract)
                d2 = mid_pool.tile([P, TILE_F], _FP16, tag="d2")
                if t % 5 == 0:
                    nc.scalar.activation(d2[:], d[:], AF.Square)
                else:
                    nc.vector.tensor_tensor(d2[:], d[:], d[:], OP.mult)

                lg = mid_pool.tile([P, 2 * TILE_F], _FP16, tag="lg")
                nc.scalar.activation(lg[:], rf[:], AF.Ln, bias=logb[:])

                dl = mid_pool.tile([P, TILE_F], _FP16, tag="dl")
                nc.vector.tensor_tensor(dl[:], lg[:, :TILE_F], lg[:, TILE_F:],
                                        OP.subtract)
                dl2 = mid_pool.tile([P, TILE_F], _FP16, tag="dl2")
                nc.vector.tensor_tensor(dl2[:], dl[:], dl[:], OP.mult)

                start = (t == 0)
                stop = (t == nt - 1)
                for j in range(J):
                    c0 = t * 2 * J + 2 * j
                    lhsT = wst[:, c0: c0 + P]
                    for cc in range(NCHUNK // J):
                        ch = j * (NCHUNK // J) + cc
                        sl = slice(ch * CHUNK, (ch + 1) * CHUNK)
                        nc.tensor.matmul(acc_d2[ch][:], lhsT, d2[:, sl],
                                         start=start, stop=stop)
                        nc.tensor.matmul(acc_dl2[ch][:], lhsT, dl2[:, sl],
                                         start=start, stop=stop)

            # Drain PSUM accumulators to a [2, 2*TILE_F] SBUF tensor, DMA out.
            out_sb = const_pool.tile([2, 2 * TILE_F], _FP32)
            for ch in range(NCHUNK):
                sl = slice(ch * CHUNK, (ch + 1) * CHUNK)
                nc.vector.tensor_copy(out_sb[:, sl], acc_d2[ch][0:2, :])
                sl2 = slice(TILE_F + ch * CHUNK, TILE_F + (ch + 1) * CHUNK)
                nc.scalar.copy(out_sb[:, sl2], acc_dl2[ch][0:2, :])
            nc.sync.dma_start(out_d[:], out_sb[:])

    return nc


_CACHE: dict = {}


def _get_nc() -> bass.Bass:
    if "nc" not in _CACHE:
        nc = _build_bass()
        nc.finalize()
        _CACHE["nc"] = nc
    return _CACHE["nc"]


def _run_device(fake: np.ndarray, real: np.ndarray, trace: bool = False):
    """Shard to 8 cores, run the bass kernel, return (per-core outs, results)."""
    nc = _get_nc()
    wst = _brow2_weights()
    fake4 = np.ascontiguousarray(fake, dtype=np.float32).reshape(B, H, W)
    real4 = np.ascontiguousarray(real, dtype=np.float32).reshape(B, H, W)
    in_maps = []
    for k in range(N_CORES):
        fs = fake4[k * IMGS_PER_CORE:(k + 1) * IMGS_PER_CORE].reshape(
            NT * P, TILE_F).astype(np.float16)
        rs = real4[k * IMGS_PER_CORE:(k + 1) * IMGS_PER_CORE].reshape(
            NT * P, TILE_F).astype(np.float16)
        in_maps.append({"real": rs, "fake": fs, "wst": wst})
    res = run_bass_kernel_spmd(nc, in_maps, list(range(N_CORES)), trace=trace)
    outs = [np.asarray(r["out"], np.float64) for r in res.results]
    return outs, res


def _finalize(outs, fake: np.ndarray, real: np.ndarray) -> np.float32:
    acol2 = ((np.arange(W, dtype=np.float64) - CX) / FX) ** 2
    sumZ = sumY = sumX = sumL = 0.0
    for o in outs:
        for ch in range(NCHUNK):
            blk = o[:, ch * CHUNK:(ch + 1) * CHUNK]
            w0 = (ch % (NCHUNK // J)) * CHUNK
            sumZ += blk[0].sum()
            sumY += blk[1].sum()
            sumX += (blk[0] * acol2[w0:w0 + CHUNK]).sum()
            sumL += o[0, TILE_F + ch * CHUNK:TILE_F + (ch + 1) * CHUNK].sum()

    # Exact corrections for elements the reference mask excludes.
    r2 = np.asarray(real, np.float32).reshape(B * H, W)
    f2 = np.asarray(fake, np.float32).reshape(B * H, W)
    inv = (r2 <= 0.0) | (r2 >= 1.0) | (f2 <= 0.0) | (f2 >= 1.0)
    n = float(B * H * W)
    if inv.any():
        iy, ix = np.nonzero(inv)
        rv = r2[iy, ix].astype(np.float64)
        fv = f2[iy, ix].astype(np.float64)
        dd2 = (rv - fv) ** 2
        ll2 = (np.log(rv + LOG_BIAS) - np.log(fv + LOG_BIAS)) ** 2
        brow2 = (((iy % H) - CY) / FY) ** 2
        sumZ -= dd2.sum()
        sumY -= (dd2 * brow2).sum()
        sumX -= (dd2 * acol2[ix]).sum()
        sumL -= ll2.sum()
        n -= float(len(iy))

    lX = np.sqrt(sumX / n)
    lY = np.sqrt(sumY / n)
    lZ = np.sqrt(sumZ / n)
    rmse_log = np.sqrt(sumL / n)
    loss = 10.0 * (rmse_log + np.abs(10.0 * (3.0 - np.exp(lX) - np.exp(lY) - np.exp(lZ))))
    return np.float32(loss)


def kernel(fake: np.ndarray, real: np.ndarray) -> np.ndarray:
    outs, _ = _run_device(fake, real, trace=False)
    return np.asarray(_finalize(outs, fake, real))


def kernel_traced(fake: np.ndarray, real: np.ndarray):
    """Like kernel() but with NTFF profiling; returns (loss, BassKernelResults)."""
    outs, res = _run_device(fake, real, trace=True)
    return np.asarray(_finalize(outs, fake, real)), res
